# revision 9
# baseline (speedup 1.0000x reference)
"""Cosine-similarity kernel (x[16384,512] vs weights[4096,512] -> [16384,4096])
on 8 Trainium2 NeuronCores, data-parallel over the x batch dim.

Strategy: cos(x, w) = (xn/|xn|)·(wn/|wn|) is a normalized GEMM.  All cheap
O(B*D) prep runs on host: normalize, scale by S, quantize to TRN fp8 e4m3,
and pre-pack transposed k-tile-paired layouts.  The device does only the
O(B*N*D) GEMM as DoubleRow fp8 matmuls (2 k-rows per partition), PSUM
accumulation, fp16 eviction, and big contiguous DMAs.

Precision: w-side is sent as a two-term fp8 expansion (w8 + s8), so the
device computes x8·(w8+s8) [+ optionally r8·w8 over half of K], which keeps
max|err|/absmax(ref) under the 2e-2 gate (measured 1.90e-2 / 1.38e-2 on the
fixed problem seed).
"""
import numpy as np
import ml_dtypes

B, D, N = 16384, 512, 4096
NCORES = 8
BS = B // NCORES          # 2048 rows per core
MT = BS // 128            # 16 m-tiles
SCALE = 128.0             # fp8 dynamic-range scale; out = psum / SCALE^2
NCHUNK = 4                # n-column chunks of 1024 for DMA/compute overlap
R_COMP = 0                # 1: add r8·w8 over K/2 (rel~1.38e-2), 0: rel~1.90e-2

E4 = ml_dtypes.float8_e4m3  # IEEE-style e4m3, max normal 240 == TRN FP8_EXP4

_cached = {}


def _build():
    import concourse.bass as bass
    import concourse.mybir as mybir
    import concourse.tile as tile
    from concourse import bacc

    F32, F16, F8 = mybir.dt.float32, mybir.dt.float16, mybir.dt.float8e4
    DR = mybir.MatmulPerfMode.DoubleRow

    nc = bacc.Bacc(None, target_bir_lowering=False)
    x8d = [nc.dram_tensor(f"x8_{g}", [128, 2, BS], F8, kind="ExternalInput")
           for g in range(2)]
    w8d = [nc.dram_tensor(f"w8_{g}", [128, 2, N], F8, kind="ExternalInput")
           for g in range(2)]
    s8d = [nc.dram_tensor(f"s8_{g}", [128, 2, N], F8, kind="ExternalInput")
           for g in range(2)]
    r8d = (nc.dram_tensor("r8", [128, 2, BS], F8, kind="ExternalInput")
           if R_COMP else None)
    outd = nc.dram_tensor("out", [BS, N], F16, kind="ExternalOutput")

    with tile.TileContext(nc) as tc:
        with (
            tc.tile_pool(name="ops", bufs=1) as ops,
            tc.tile_pool(name="ostage", bufs=10) as ostage,
            tc.tile_pool(name="mmps", bufs=6, space="PSUM") as mmps,
            tc.tile_pool(name="wmps", bufs=1, space="PSUM") as wmps,
        ):
            x8t = [ops.tile([128, 2, BS], F8, name=f"x8t{g}") for g in range(2)]
            w8t = [ops.tile([128, 2, N], F8, name=f"w8t{g}") for g in range(2)]
            s8t = [ops.tile([128, 2, N], F8, name=f"s8t{g}") for g in range(2)]
            r8t = ops.tile([128, 2, BS], F8, name="r8t") if R_COMP else None
            zpad = ops.tile([128, 2, 512], F8, name="zpad")
            wps = wmps.tile([128, 512], F32, name="wps")

            # Warm up the PE p-state during the input-DMA window: ~15 dummy
            # DoubleRow matmuls on a zeroed tile so the real matmuls start at
            # the full 2.4 GHz clock instead of paying the 3us ramp.
            nc.vector.memset(zpad[:], 0)
            for _ in range(15):
                nc.tensor.matmul(wps[:], zpad[:, :, :128], zpad[:],
                                 start=True, stop=True,
                                 perf_mode=mybir.MatmulPerfMode.DoubleRow)

            # x-side first (needed by every block), then w/s per n-chunk so
            # the first chunk's matmuls start early.  The x DMA is split so
            # the first m-tiles' stationary data lands sooner.
            xs0 = slice(0, 512)
            xs1 = slice(512, BS)
            for g in range(2):
                nc.sync.dma_start(x8t[g][:, :, xs0], x8d[g][:, :, xs0])
            for c in range(NCHUNK):
                cs = slice(c * (N // NCHUNK), (c + 1) * (N // NCHUNK))
                for g in range(2):
                    nc.sync.dma_start(w8t[g][:, :, cs], w8d[g][:, :, cs])
                    nc.sync.dma_start(s8t[g][:, :, cs], s8d[g][:, :, cs])
                if c == 0:
                    for g in range(2):
                        nc.sync.dma_start(x8t[g][:, :, xs1], x8d[g][:, :, xs1])
                    if R_COMP:
                        nc.sync.dma_start(r8t[:], r8d[:])

            ev = 0
            for c in range(NCHUNK):
                for m in range(MT):
                    ms = slice(m * 128, (m + 1) * 128)
                    ot = ostage.tile([128, N // NCHUNK], F16, name="ot", tag="ot")
                    for nbh in range(2):
                        nb = c * 2 + nbh
                        ns = slice(nb * 512, (nb + 1) * 512)
                        pm = mmps.tile([128, 512], F32, name="pm", tag="pm")
                        nc.tensor.matmul(pm[:], x8t[0][:, :, ms], w8t[0][:, :, ns],
                                         start=True, stop=False, perf_mode=DR)
                        nc.tensor.matmul(pm[:], x8t[1][:, :, ms], w8t[1][:, :, ns],
                                         start=False, stop=False, perf_mode=DR)
                        nc.tensor.matmul(pm[:], x8t[0][:, :, ms], s8t[0][:, :, ns],
                                         start=False, stop=False, perf_mode=DR)
                        nc.tensor.matmul(pm[:], x8t[1][:, :, ms], s8t[1][:, :, ns],
                                         start=False, stop=(not R_COMP),
                                         perf_mode=DR)
                        if R_COMP:
                            nc.tensor.matmul(pm[:], r8t[:, :, ms],
                                             w8t[0][:, :, ns],
                                             start=False, stop=True, perf_mode=DR)
                        dst = ot[:, nbh * 512:(nbh + 1) * 512]
                        if ev % 2 == 0:
                            nc.scalar.copy(dst, pm[:])
                        else:
                            nc.vector.tensor_copy(dst, pm[:])
                        ev += 1
                        last = (c == NCHUNK - 1 and m == MT - 1)
                        if last:
                            # final tile: store each 512-col half as soon as
                            # its eviction lands, shortening the tail chain
                            nc.sync.dma_start(
                                outd[ms, nb * 512:(nb + 1) * 512], dst)
                    if not last:
                        nc.sync.dma_start(
                            outd[ms, c * (N // NCHUNK):(c + 1) * (N // NCHUNK)],
                            ot[:])
    nc.compile()
    return nc


def _q8(a):
    return np.clip(a, -240.0, 240.0).astype(E4)


def _pack(t8):
    """[rows, 512] fp8 -> per k-group g: [128 (d'), 2 (slot j), rows] with
    d = g*256 + j*128 + d', matching the DoubleRow operand layout."""
    a = np.ascontiguousarray(t8.T).reshape(2, 2, 128, t8.shape[0])
    a = a.transpose(0, 2, 1, 3)
    return [np.ascontiguousarray(a[g]) for g in range(2)]


def kernel(x: np.ndarray, weights: np.ndarray) -> np.ndarray:
    from concourse.bass_utils import run_bass_kernel_spmd

    if "nc" not in _cached:
        _cached["nc"] = _build()
    nc = _cached["nc"]

    x = np.ascontiguousarray(x, dtype=np.float32)
    w = np.ascontiguousarray(weights, dtype=np.float32)
    xn = x * (SCALE / np.maximum(np.linalg.norm(x, axis=1, keepdims=True), 1e-8))
    wn = w * (SCALE / np.maximum(np.linalg.norm(w, axis=1, keepdims=True), 1e-8))

    x8 = _q8(xn)
    w8 = _q8(wn)
    s8 = _q8(wn - w8.astype(np.float32))
    xp = _pack(x8)
    wp = _pack(w8)
    sp = _pack(s8)
    if R_COMP:
        r8 = _q8(xn - x8.astype(np.float32))
        rp = _pack(r8)

    in_maps = []
    for i in range(NCORES):
        bs = slice(i * BS, (i + 1) * BS)
        m = {
            "x8_0": np.ascontiguousarray(xp[0][:, :, bs]),
            "x8_1": np.ascontiguousarray(xp[1][:, :, bs]),
            "w8_0": wp[0], "w8_1": wp[1],
            "s8_0": sp[0], "s8_1": sp[1],
        }
        if R_COMP:
            m["r8"] = np.ascontiguousarray(rp[0][:, :, bs])
        in_maps.append(m)

    res = run_bass_kernel_spmd(nc, in_maps, list(range(NCORES)))
    out = np.concatenate([res.results[i]["out"] for i in range(NCORES)], axis=0)
    return out.astype(np.float32) * np.float32(1.0 / (SCALE * SCALE))


# revision 10
# speedup vs baseline: 1.0378x; 1.0378x over previous
"""Cosine-similarity kernel (x[16384,512] vs weights[4096,512] -> [16384,4096])
on 8 Trainium2 NeuronCores, data-parallel over the x batch dim.

Strategy: cos(x, w) = (xn/|xn|)·(wn/|wn|) is a normalized GEMM.  All cheap
O(B*D) prep runs on host: normalize, scale by S, quantize to TRN fp8 e4m3,
and pre-pack transposed k-tile-paired layouts.  The device does only the
O(B*N*D) GEMM as DoubleRow fp8 matmuls (2 k-rows per partition), PSUM
accumulation, fp16 eviction, and big contiguous DMAs.

Precision: w-side is sent as a two-term fp8 expansion (w8 + s8), so the
device computes x8·(w8+s8) [+ optionally r8·w8 over half of K], which keeps
max|err|/absmax(ref) under the 2e-2 gate (measured 1.90e-2 / 1.38e-2 on the
fixed problem seed).
"""
import numpy as np
import ml_dtypes

B, D, N = 16384, 512, 4096
NCORES = 8
BS = B // NCORES          # 2048 rows per core
MT = BS // 128            # 16 m-tiles
SCALE = 128.0             # fp8 dynamic-range scale; out = psum / SCALE^2
NCHUNK = 4                # n-column chunks of 1024 for DMA/compute overlap
R_COMP = 0                # 1: add r8·w8 over K/2 (rel~1.38e-2), 0: rel~1.90e-2

E4 = ml_dtypes.float8_e4m3  # IEEE-style e4m3, max normal 240 == TRN FP8_EXP4

_cached = {}


def _build():
    import concourse.bass as bass
    import concourse.mybir as mybir
    import concourse.tile as tile
    from concourse import bacc

    F32, F16, F8 = mybir.dt.float32, mybir.dt.float16, mybir.dt.float8e4
    DR = mybir.MatmulPerfMode.DoubleRow

    nc = bacc.Bacc(None, target_bir_lowering=False)
    x8d = [nc.dram_tensor(f"x8_{g}", [128, 2, BS], F8, kind="ExternalInput")
           for g in range(2)]
    w8d = [nc.dram_tensor(f"w8_{g}", [128, 2, N], F8, kind="ExternalInput")
           for g in range(2)]
    s8d = [nc.dram_tensor(f"s8_{g}", [128, 2, N], F8, kind="ExternalInput")
           for g in range(2)]
    r8d = (nc.dram_tensor("r8", [128, 2, BS], F8, kind="ExternalInput")
           if R_COMP else None)
    outd = nc.dram_tensor("out", [BS, N], F16, kind="ExternalOutput")

    with tile.TileContext(nc) as tc:
        with (
            tc.tile_pool(name="ops", bufs=1) as ops,
            tc.tile_pool(name="ostage", bufs=20) as ostage,
            tc.tile_pool(name="mmps", bufs=6, space="PSUM") as mmps,
            tc.tile_pool(name="wmps", bufs=1, space="PSUM") as wmps,
        ):
            x8t = [ops.tile([128, 2, BS], F8, name=f"x8t{g}") for g in range(2)]
            w8t = [ops.tile([128, 2, N], F8, name=f"w8t{g}") for g in range(2)]
            s8t = [ops.tile([128, 2, N], F8, name=f"s8t{g}") for g in range(2)]
            r8t = ops.tile([128, 2, BS], F8, name="r8t") if R_COMP else None
            zpad = ops.tile([128, 2, 512], F8, name="zpad")
            wps = wmps.tile([128, 512], F32, name="wps")

            # Warm up the PE p-state during the input-DMA window: ~15 dummy
            # DoubleRow matmuls on a zeroed tile so the real matmuls start at
            # the full 2.4 GHz clock instead of paying the 3us ramp.
            nc.vector.memset(zpad[:], 0)
            for _ in range(15):
                nc.tensor.matmul(wps[:], zpad[:, :, :128], zpad[:],
                                 start=True, stop=True,
                                 perf_mode=mybir.MatmulPerfMode.DoubleRow)

            # x-side first (needed by every block), then w/s per n-chunk so
            # the first chunk's matmuls start early.  The x DMA is split so
            # the first m-tiles' stationary data lands sooner.
            xs0 = slice(0, 512)
            xs1 = slice(512, BS)
            for g in range(2):
                nc.sync.dma_start(x8t[g][:, :, xs0], x8d[g][:, :, xs0])
            for c in range(NCHUNK):
                cs = slice(c * (N // NCHUNK), (c + 1) * (N // NCHUNK))
                for g in range(2):
                    nc.sync.dma_start(w8t[g][:, :, cs], w8d[g][:, :, cs])
                    nc.sync.dma_start(s8t[g][:, :, cs], s8d[g][:, :, cs])
                if c == 0:
                    for g in range(2):
                        nc.sync.dma_start(x8t[g][:, :, xs1], x8d[g][:, :, xs1])
                    if R_COMP:
                        nc.sync.dma_start(r8t[:], r8d[:])

            ev = 0
            for c in range(NCHUNK):
                for m in range(MT):
                    ms = slice(m * 128, (m + 1) * 128)
                    ot = ostage.tile([128, N // NCHUNK], F16, name="ot", tag="ot")
                    for nbh in range(2):
                        nb = c * 2 + nbh
                        ns = slice(nb * 512, (nb + 1) * 512)
                        pm = mmps.tile([128, 512], F32, name="pm", tag="pm")
                        nc.tensor.matmul(pm[:], x8t[0][:, :, ms], w8t[0][:, :, ns],
                                         start=True, stop=False, perf_mode=DR)
                        nc.tensor.matmul(pm[:], x8t[1][:, :, ms], w8t[1][:, :, ns],
                                         start=False, stop=False, perf_mode=DR)
                        nc.tensor.matmul(pm[:], x8t[0][:, :, ms], s8t[0][:, :, ns],
                                         start=False, stop=False, perf_mode=DR)
                        nc.tensor.matmul(pm[:], x8t[1][:, :, ms], s8t[1][:, :, ns],
                                         start=False, stop=(not R_COMP),
                                         perf_mode=DR)
                        if R_COMP:
                            nc.tensor.matmul(pm[:], r8t[:, :, ms],
                                             w8t[0][:, :, ns],
                                             start=False, stop=True, perf_mode=DR)
                        dst = ot[:, nbh * 512:(nbh + 1) * 512]
                        if ev % 2 == 0:
                            nc.scalar.copy(dst, pm[:])
                        else:
                            nc.vector.tensor_copy(dst, pm[:])
                        ev += 1
                        last = (c == NCHUNK - 1 and m == MT - 1)
                        if last:
                            # final tile: store each 512-col half as soon as
                            # its eviction lands, shortening the tail chain
                            nc.sync.dma_start(
                                outd[ms, nb * 512:(nb + 1) * 512], dst)
                    if not last:
                        nc.sync.dma_start(
                            outd[ms, c * (N // NCHUNK):(c + 1) * (N // NCHUNK)],
                            ot[:])
    nc.compile()
    return nc


def _q8(a):
    return np.clip(a, -240.0, 240.0).astype(E4)


def _pack(t8):
    """[rows, 512] fp8 -> per k-group g: [128 (d'), 2 (slot j), rows] with
    d = g*256 + j*128 + d', matching the DoubleRow operand layout."""
    a = np.ascontiguousarray(t8.T).reshape(2, 2, 128, t8.shape[0])
    a = a.transpose(0, 2, 1, 3)
    return [np.ascontiguousarray(a[g]) for g in range(2)]


def kernel(x: np.ndarray, weights: np.ndarray) -> np.ndarray:
    from concourse.bass_utils import run_bass_kernel_spmd

    if "nc" not in _cached:
        _cached["nc"] = _build()
    nc = _cached["nc"]

    x = np.ascontiguousarray(x, dtype=np.float32)
    w = np.ascontiguousarray(weights, dtype=np.float32)
    xn = x * (SCALE / np.maximum(np.linalg.norm(x, axis=1, keepdims=True), 1e-8))
    wn = w * (SCALE / np.maximum(np.linalg.norm(w, axis=1, keepdims=True), 1e-8))

    x8 = _q8(xn)
    w8 = _q8(wn)
    s8 = _q8(wn - w8.astype(np.float32))
    xp = _pack(x8)
    wp = _pack(w8)
    sp = _pack(s8)
    if R_COMP:
        r8 = _q8(xn - x8.astype(np.float32))
        rp = _pack(r8)

    in_maps = []
    for i in range(NCORES):
        bs = slice(i * BS, (i + 1) * BS)
        m = {
            "x8_0": np.ascontiguousarray(xp[0][:, :, bs]),
            "x8_1": np.ascontiguousarray(xp[1][:, :, bs]),
            "w8_0": wp[0], "w8_1": wp[1],
            "s8_0": sp[0], "s8_1": sp[1],
        }
        if R_COMP:
            m["r8"] = np.ascontiguousarray(rp[0][:, :, bs])
        in_maps.append(m)

    res = run_bass_kernel_spmd(nc, in_maps, list(range(NCORES)))
    out = np.concatenate([res.results[i]["out"] for i in range(NCORES)], axis=0)
    return out.astype(np.float32) * np.float32(1.0 / (SCALE * SCALE))
